# revision 6
# baseline (speedup 1.0000x reference)
"""Trainium2 Bass kernel for the DfOp deep-filtering module.

out[b, t, f<96]  = sum_{k=0..4} coefs[b, k, t, f] (*) spec[b, t-4+k, f]   (complex mult)
out[b, t, f>=96] = spec[b, t, f]                                          (passthrough)

Sharding: data-parallel over batch B=8 -> one batch element per NeuronCore.

Per-core layout: partition p holds the 32-timestep block t in [32p, 32p+32),
processed in chunks of [4, 10, 10, 8] timesteps (small first chunk so the
compute pipeline starts early).  Spec is loaded as FULL 962-float DRAM rows
(one contiguous run per partition per chunk -> 128 descriptors per DMA,
near-peak HBM streaming).  The filtered lo-band is written back IN PLACE
into the same tile (the hi-band passthrough then never moves on-chip) and
the tile is stored back as full rows.

Each chunk materializes a packed "window" tile holding the chunk's lo-band
plus a 4-slot halo (chained from the previous chunk's window), so the causal
5-tap window is a pure free-dim offset and every DVE product is one unsplit
instruction.

Compute (all fp32, bit-exact accumulation):
  DVE: per tap, 4 real products (rr, -ii via fused scalar_tensor_tensor,
       ri, ir) + pair-combines D = rr - ii, E = ri + ir.
  PE : accumulates the 5 taps' D (resp. E) into PSUM with identity-weight
       matmuls (exact fp32 PSUM accumulate).
  ACT: window fills/extracts, PSUM->lo-band interleave.
  DMA: loads + last-chunk hi-band store on the Sync HWDGE ring; row stores
       on the Scalar HWDGE ring.
"""

import numpy as np

import concourse.bacc as bacc
import concourse.mybir as mybir
from concourse.tile import TileContext
from concourse.bass_utils import run_bass_kernel_spmd

B = 8          # batch / cores
T = 4096       # time steps
F = 481        # total freq bins
NF = 96        # deep-filtered freq bins
FS = 5         # frame size (causal taps)
HL = FS - 1    # halo slots (4)
ROW = 2 * F    # floats per DRAM time row        (962)
U = 2 * NF     # lo-band floats per time row     (192)
P = 128        # partitions
TB = T // P    # timesteps per partition block   (32)
SIZES = [4, 10, 10, 8]        # per-chunk timesteps (sum = TB)
OFFS = [0, 4, 14, 24]         # cumulative offsets
WMAX = (max(SIZES) + HL) * U  # window tile cols

_nc_cache = None


def _mm_ranges(cw):
    return [(a, min(a + 512, cw)) for a in range(0, cw, 512)]


def _body(nc, tc, spec_d, coefs_d, ident_d, out_d):
    f32 = mybir.dt.float32
    mult = mybir.AluOpType.mult

    specv = spec_d.rearrange("(q i) u -> q i u", i=TB)          # [128, 32, 962]
    outv = out_d.rearrange("(q i) u -> q i u", i=TB)
    coefv = [coefs_d[k].rearrange("(q i) u -> q i u", i=TB) for k in range(FS)]

    with (
        tc.tile_pool(name="const", bufs=1) as cpool,
        tc.tile_pool(name="spec", bufs=2) as spool,
        tc.tile_pool(name="win", bufs=2) as wpool,
        tc.tile_pool(name="coef", bufs=7) as kpool,
        tc.tile_pool(name="prod", bufs=5) as ppool,
        tc.tile_pool(name="psum", bufs=2, space="PSUM") as pspool,
    ):
        ident_sb = cpool.tile([P, P], f32)
        nc.scalar.dma_start(out=ident_sb[:], in_=ident_d)

        prev_w = None
        prev_ti = None
        for ch, (i0, TI) in enumerate(zip(OFFS, SIZES)):
            CW = TI * NF

            stile = spool.tile([P, TI * ROW], f32, tag="spec")
            nc.sync.dma_start(
                out=stile[:],
                in_=specv[:, i0:i0 + TI, :].rearrange("q i u -> q (i u)"),
            )
            ctiles = []
            for k in range(FS):
                ct = kpool.tile([P, TI * U], f32, tag="coef")
                nc.sync.dma_start(
                    out=ct[:],
                    in_=coefv[k][:, i0:i0 + TI, :].rearrange("q i u -> q (i u)"),
                )
                ctiles.append(ct)

            # window tile: [halo(4) | chunk lo-band(TI)] packed, 192 floats/slot
            wtile = wpool.tile([P, WMAX], f32, tag="win")
            if ch == 0:
                nc.vector.memzero(wtile[0:1, 0:HL * U])
                # scalar ring: idle at kernel start, so the many-descriptor
                # halo gather completes immediately instead of straggling
                # behind the big spec loads on the sync ring
                nc.scalar.dma_start(
                    out=wtile[:].rearrange("p (j u) -> p j u", u=U)[1:P, 0:HL],
                    in_=specv[0:P - 1, TB - HL:TB, 0:U],
                )
            else:
                nc.scalar.copy(
                    out=wtile[:, 0:HL * U],
                    in_=prev_w[:, prev_ti * U:(prev_ti + HL) * U],
                )
            sfc = stile[:].rearrange("p (i f c) -> p i f c", f=F, c=2)
            nc.scalar.copy(
                out=wtile[:].rearrange("p (j u) -> p j u", u=U)[:, HL:HL + TI],
                in_=sfc[:, :, 0:NF, :].rearrange("p i f c -> p i (f c)"),
            )
            wfc = wtile[:].rearrange("p (j f c) -> p j f c", f=NF, c=2)

            # last chunk: store the untouched hi-band early (overlaps compute),
            # so the final row store only covers the lo-band
            if ch == len(SIZES) - 1:
                nc.sync.dma_start(
                    out=outv[:, i0:i0 + TI, U:ROW],
                    in_=sfc[:, :, NF:F, :].rearrange("p i f c -> p i (f c)"),
                )

            ps_re = pspool.tile([P, CW], f32, tag="psre")
            ps_im = pspool.tile([P, CW], f32, tag="psim")

            for k in range(FS):
                s_re = wfc[:, k:k + TI, :, 0]                 # [128, TI, 96]
                s_im = wfc[:, k:k + TI, :, 1]
                cvfc = ctiles[k][:].rearrange("p (i f c) -> p i f c", f=NF, c=2)
                c_re = cvfc[:, :, :, 0]
                c_im = cvfc[:, :, :, 1]

                prr = ppool.tile([P, CW], f32, tag="prod")
                pii = ppool.tile([P, CW], f32, tag="prod")
                pri = ppool.tile([P, CW], f32, tag="prod")
                pir = ppool.tile([P, CW], f32, tag="prod")
                pv = lambda t: t[:].rearrange("p (i f) -> p i f", f=NF)

                nc.vector.tensor_mul(out=pv(prr), in0=s_re, in1=c_re)
                nc.vector.scalar_tensor_tensor(
                    out=pv(pii), in0=s_im, scalar=-1.0, in1=c_im,
                    op0=mult, op1=mult,
                )
                nc.vector.tensor_mul(out=pv(pri), in0=s_re, in1=c_im)
                nc.vector.tensor_mul(out=pv(pir), in0=s_im, in1=c_re)
                nc.vector.tensor_add(out=prr[:], in0=prr[:], in1=pii[:])  # D
                nc.vector.tensor_add(out=pri[:], in0=pri[:], in1=pir[:])  # E

                for src, ps in ((prr, ps_re), (pri, ps_im)):
                    for a, b in _mm_ranges(CW):
                        nc.tensor.matmul(
                            ps[:, a:b], ident_sb[:], src[:, a:b],
                            start=(k == 0), stop=(k == FS - 1),
                        )

            # interleave PSUM into the tile's lo band (in place), store rows
            psv = lambda t: t[:].rearrange("p (i f) -> p i f", f=NF)
            nc.scalar.copy(out=sfc[:, :, 0:NF, 0], in_=psv(ps_re))
            nc.scalar.copy(out=sfc[:, :, 0:NF, 1], in_=psv(ps_im))
            if ch == len(SIZES) - 1:
                nc.scalar.dma_start(
                    out=outv[:, i0:i0 + TI, 0:U],
                    in_=sfc[:, :, 0:NF, :].rearrange("p i f c -> p i (f c)"),
                )
            else:
                nc.scalar.dma_start(
                    out=outv[:, i0:i0 + TI, :].rearrange("q i u -> q (i u)"),
                    in_=stile[:],
                )

            prev_w, prev_ti = wtile, TI


def _build_nc():
    nc = bacc.Bacc("TRN2", target_bir_lowering=False, debug=False, num_devices=B)
    f32 = mybir.dt.float32
    spec_d = nc.dram_tensor("spec", [T, ROW], f32, kind="ExternalInput").ap()
    coefs_d = nc.dram_tensor("coefs", [FS, T, U], f32, kind="ExternalInput").ap()
    ident_d = nc.dram_tensor("ident", [P, P], f32, kind="ExternalInput").ap()
    out_d = nc.dram_tensor("out", [T, ROW], f32, kind="ExternalOutput").ap()
    with TileContext(nc) as tc:
        _body(nc, tc, spec_d, coefs_d, ident_d, out_d)
    nc.compile()
    return nc


def _in_maps(spec, coefs):
    spec = np.asarray(spec, dtype=np.float32)
    coefs = np.asarray(coefs, dtype=np.float32)
    ident = np.eye(P, dtype=np.float32)
    maps = []
    for b in range(B):
        maps.append({
            "spec": np.ascontiguousarray(spec[b, 0].reshape(T, ROW)),
            "coefs": np.ascontiguousarray(coefs[b].reshape(FS, T, U)),
            "ident": ident,
        })
    return maps


def kernel(spec, coefs):
    global _nc_cache
    if _nc_cache is None:
        _nc_cache = _build_nc()
    res = run_bass_kernel_spmd(_nc_cache, _in_maps(spec, coefs),
                               core_ids=list(range(B)))
    return np.stack(
        [res.results[b]["out"].reshape(1, T, F, 2) for b in range(B)]
    ).astype(np.float32)
